# revision 4
# baseline (speedup 1.0000x reference)
"""Trainium2 Bass kernel for GQA attention (B=2, S=2048, D=1024, 16 q heads,
4 kv heads, head_dim 64, RoPE, causal).

Sharding: 8 cores = 2 (batch) x 4 (kv-head groups). Each core computes, for
its batch b and kv group g: the 4 query heads of group g + 1 kv head, plus the
partial output projection y_partial = attn_out_g @ wo[:, g_cols].T.  The host
unshard step sums the 4 partials per batch (the canonical all-reduce of
row-parallel TP, done on host since each core's output is already needed
host-side).

Device-side layout choices (all matmuls contract over the partition dim):
  - x is fed transposed (D on partitions) so QKV projections produce Q^T/K^T
    (head_dim on partitions, seq on free dim).
  - RoPE: wq/wk rows are permuted on host so Q^T rows 0-31 are the "real"
    pair lanes and 32-63 the "imag" lanes; RoPE is then 6 elementwise DVE ops
    on contiguous partition slices.  (Permutation cancels in Q.K^T.)
  - Scores are computed as S^T (keys on partitions, queries on free):
    lhsT = K^T block, rhs = Q^T block.  Softmax needs no max-subtraction
    (|scores/8| <~ 3), so exp runs directly on the PSUM scores; the
    denominator is produced by an extra ones-row in the V stationary
    (out row 64 of the PV matmul = sum_l P^T[l, q]).
  - Causal mask: matmuls are only emitted for the lower-triangle blocks; the
    128x128 diagonal blocks are masked multiplicatively (tri mask) after exp.
  - Normalization (divide by denominator, which lives along the free dim) is
    done with vector ops against a gpsimd partition_broadcast of 1/denom.
"""

import sys

sys.path.insert(0, "/opt/trn_rl_repo")

from contextlib import ExitStack

import ml_dtypes
import numpy as np

import concourse.bass as bass
import concourse.mybir as mybir
import concourse.tile as tile
from concourse import bacc
from concourse.masks import make_identity

# ---------------------------------------------------------------- constants
B, S, D = 2, 2048, 1024
HD = 64
HALF = HD // 2
HKV = 4          # kv heads total
NH = 4           # q heads per core (= NREP)
KVD = HKV * HD   # 256
GO = NH * HD     # 256 output features per group
N_CORES = 8

SB = 512         # q superblock (matmul free dim)
NQS = S // SB    # 4 q superblocks
NLB = S // 128   # 16 key blocks of 128
KCH = D // 128   # 8 contraction chunks for projections

F32 = mybir.dt.float32
BF16 = mybir.dt.bfloat16
SCALE = 1.0 / 8.0  # 1/sqrt(64)

_CACHE = {}


# ---------------------------------------------------------------- builder
def _build():
    nc = bacc.Bacc("TRN2", target_bir_lowering=False, debug=False,
                   enable_asserts=False, num_devices=N_CORES)

    xt_d = nc.dram_tensor("xt", [D, S], BF16, kind="ExternalInput").ap()
    wqt_d = nc.dram_tensor("wqt", [D, GO], BF16, kind="ExternalInput").ap()
    wkt_d = nc.dram_tensor("wkt", [D, HD], BF16, kind="ExternalInput").ap()
    wvt_d = nc.dram_tensor("wvt", [D, HD], BF16, kind="ExternalInput").ap()
    wot_d = nc.dram_tensor("wot", [GO, D], BF16, kind="ExternalInput").ap()
    cost_d = nc.dram_tensor("cost", [HALF, S], F32, kind="ExternalInput").ap()
    sint_d = nc.dram_tensor("sint", [HALF, S], F32, kind="ExternalInput").ap()
    tri_d = nc.dram_tensor("tri", [128, 128], BF16, kind="ExternalInput").ap()
    out_d = nc.dram_tensor("out", [S, D], F32, kind="ExternalOutput").ap()

    with TileKernel(nc) as tk:
        tk.build(xt_d, wqt_d, wkt_d, wvt_d, wot_d, cost_d, sint_d, tri_d, out_d)

    nc.compile()
    return nc


class TileKernel:
    def __init__(self, nc):
        self.nc = nc
        self.ctx = ExitStack()
        self.tc = None

    def __enter__(self):
        self.ctx.__enter__()
        self.tc = self.ctx.enter_context(tile.TileContext(self.nc))
        return self

    def __exit__(self, *a):
        return self.ctx.__exit__(*a)

    def build(self, xt_d, wqt_d, wkt_d, wvt_d, wot_d, cost_d, sint_d, tri_d, out_d):
        nc = self.nc
        tc = self.tc
        ctx = self.ctx

        perm = ctx.enter_context(tc.tile_pool(name="perm", bufs=1))
        pexp = ctx.enter_context(tc.tile_pool(name="pexp", bufs=6))
        ptmp = ctx.enter_context(tc.tile_pool(name="ptmp", bufs=4))
        pout = ctx.enter_context(tc.tile_pool(name="pout", bufs=3))
        pp_mm = ctx.enter_context(tc.tile_pool(name="ppmm", bufs=3, space="PSUM"))
        pp_pv = ctx.enter_context(tc.tile_pool(name="pppv", bufs=2, space="PSUM"))
        pp_tr = ctx.enter_context(tc.tile_pool(name="pptr", bufs=2, space="PSUM"))

        # ---------------- persistent SBUF tensors
        xt_sb = perm.tile([128, KCH, S], BF16, tag="xt")
        wqt_sb = perm.tile([128, KCH, GO], BF16, tag="wqt")
        wkt_sb = perm.tile([128, KCH, HD], BF16, tag="wkt")
        wvt_sb = perm.tile([128, KCH, HD], BF16, tag="wvt")
        wot_sb = perm.tile([128, 2, D], BF16, tag="wot")
        cost_sb = perm.tile([HALF, S], F32, tag="cost")
        sint_sb = perm.tile([HALF, S], F32, tag="sint")
        tri_sb = perm.tile([128, 128], BF16, tag="tri")
        ident = perm.tile([64, 64], BF16, tag="ident")
        qT_sb = perm.tile([64, NH, S], BF16, tag="qT")       # [hd, h, s]
        kT_sb = perm.tile([64, S], BF16, tag="kT")           # [hd, s]
        v_sb = perm.tile([128, NLB, HD + 1], BF16, tag="v")  # [l, lb, hd|1]
        att_sb = perm.tile([128, 2, S], BF16, tag="att")     # [o%128, o//128, s]

        # ---------------- input DMAs
        nc.sync.dma_start(wqt_sb[:], wqt_d.rearrange("(kc p) m -> p kc m", p=128))
        nc.sync.dma_start(wkt_sb[:], wkt_d.rearrange("(kc p) m -> p kc m", p=128))
        nc.sync.dma_start(wvt_sb[:], wvt_d.rearrange("(kc p) m -> p kc m", p=128))
        nc.sync.dma_start(wot_sb[:], wot_d.rearrange("(oc p) d -> p oc d", p=128))
        nc.sync.dma_start(cost_sb[:], cost_d)
        nc.sync.dma_start(sint_sb[:], sint_d)
        nc.sync.dma_start(tri_sb[:], tri_d)
        xt_r = xt_d.rearrange("(kc p) s -> p kc s", p=128)
        for si in range(NQS):
            nc.sync.dma_start(xt_sb[:, :, si * SB:(si + 1) * SB],
                              xt_r[:, :, si * SB:(si + 1) * SB])
        make_identity(nc, ident[:])

        # ---------------- helper: RoPE on one 32-lane pair set
        def rope(dst_r, dst_i, src_r, src_i, cols):
            c = cost_sb[:, cols]
            s = sint_sb[:, cols]
            t0 = ptmp.tile([HALF, SB], F32, tag="ropet0")
            t1 = ptmp.tile([HALF, SB], F32, tag="ropet1")
            nc.vector.tensor_mul(t0[:], src_r, c)
            nc.vector.tensor_mul(t1[:], src_i, s)
            nc.vector.tensor_sub(dst_r, t0[:], t1[:])
            t2 = ptmp.tile([HALF, SB], F32, tag="ropet2")
            t3 = ptmp.tile([HALF, SB], F32, tag="ropet3")
            nc.vector.tensor_mul(t2[:], src_r, s)
            nc.vector.tensor_mul(t3[:], src_i, c)
            nc.vector.tensor_add(dst_i, t2[:], t3[:])

        # ---------------- Q projection + RoPE
        for mi in range(2):           # two 128-row chunks = heads (2mi, 2mi+1)
            for si in range(NQS):
                cols = slice(si * SB, (si + 1) * SB)
                ps = pp_mm.tile([128, SB], F32, tag="mm")
                for kc in range(KCH):
                    nc.tensor.matmul(
                        ps[:], wqt_sb[:, kc, mi * 128:(mi + 1) * 128],
                        xt_sb[:, kc, cols], start=(kc == 0), stop=(kc == KCH - 1))
                for h2 in range(2):
                    h = 2 * mi + h2
                    b0 = h2 * 64
                    rope(qT_sb[0:HALF, h, cols], qT_sb[HALF:64, h, cols],
                         ps[b0:b0 + HALF, :], ps[b0 + HALF:b0 + 64, :], cols)

        # ---------------- K projection + RoPE
        for si in range(NQS):
            cols = slice(si * SB, (si + 1) * SB)
            psf = pp_mm.tile([128, SB], F32, tag="mm")
            ps = psf[0:64]
            for kc in range(KCH):
                nc.tensor.matmul(ps[:], wkt_sb[:, kc, :], xt_sb[:, kc, cols],
                                 start=(kc == 0), stop=(kc == KCH - 1))
            rope(kT_sb[0:HALF, cols], kT_sb[HALF:64, cols],
                 ps[0:HALF, :], ps[HALF:64, :], cols)

        # ---------------- V projection (V^T), then transpose to natural V
        nc.vector.memset(v_sb[:, :, HD:HD + 1], 1.0)  # ones column -> denom
        for si in range(NQS):
            cols = slice(si * SB, (si + 1) * SB)
            psf = pp_mm.tile([128, SB], F32, tag="mm")
            ps = psf[0:64]
            for kc in range(KCH):
                nc.tensor.matmul(ps[:], wvt_sb[:, kc, :], xt_sb[:, kc, cols],
                                 start=(kc == 0), stop=(kc == KCH - 1))
            vt = ptmp.tile([64, SB], BF16, tag="vtstage")
            nc.vector.tensor_copy(vt[:], ps[:])
            for j in range(SB // 128):
                lb = si * (SB // 128) + j
                pt = pp_tr.tile([128, 64], BF16, tag="tr")
                nc.tensor.transpose(pt[:], vt[:, j * 128:(j + 1) * 128], ident[:])
                nc.vector.tensor_copy(v_sb[:, lb, 0:HD], pt[:])

        # ---------------- attention + output projection, per q superblock
        for qs in range(NQS):
            qcols = slice(qs * SB, (qs + 1) * SB)
            nlb = 4 * qs + 4          # key blocks needed (block-causal)
            for h in range(NH):
                po = pp_pv.tile([HD + 1, SB], F32, tag="pv")
                for lb in range(nlb):
                    j = lb - 4 * qs   # >=0 on the diagonal superblock
                    sp = pp_mm.tile([128, SB], F32, tag="mm")
                    nc.tensor.matmul(sp[:], kT_sb[:, lb * 128:(lb + 1) * 128],
                                     qT_sb[:, h, qcols], start=True, stop=True)
                    pe = pexp.tile([128, SB], BF16, tag="pexp")
                    if j < 0:
                        nc.scalar.activation(pe[:], sp[:],
                                             mybir.ActivationFunctionType.Exp,
                                             scale=SCALE)
                        nc.tensor.matmul(po[:], v_sb[:, lb, :], pe[:],
                                         start=(lb == 0), stop=(lb == nlb - 1))
                    else:
                        vcols = slice(j * 128, SB)
                        nc.scalar.activation(pe[:, vcols], sp[:, vcols],
                                             mybir.ActivationFunctionType.Exp,
                                             scale=SCALE)
                        dcols = slice(j * 128, (j + 1) * 128)
                        nc.vector.tensor_mul(pe[:, dcols], pe[:, dcols], tri_sb[:])
                        nc.tensor.matmul(po[:, vcols], v_sb[:, lb, :], pe[:, vcols],
                                         start=(lb == 0), stop=(lb == nlb - 1))
                # normalize: att = po[0:64] * (1/denom) broadcast over partitions
                rc = ptmp.tile([1, SB], F32, tag="recip")
                nc.vector.reciprocal(rc[:], po[HD:HD + 1, :])
                rb = ptmp.tile([64, SB], F32, tag="recipb")
                nc.gpsimd.partition_broadcast(rb[:], rc[:])
                nc.vector.tensor_mul(
                    att_sb[(h % 2) * 64:(h % 2) * 64 + 64, h // 2, qcols],
                    po[0:HD, :], rb[:])

            # ---- output projection for this q range
            for sj in range(SB // 128):
                si = qs * 4 + sj
                scols = slice(si * 128, (si + 1) * 128)
                ysb = pout.tile([128, D], F32, tag="ysb")
                for dh in range(2):
                    yp = pp_mm.tile([128, 512], F32, tag="mm")
                    for oc in range(2):
                        nc.tensor.matmul(
                            yp[:], att_sb[:, oc, scols],
                            wot_sb[:, oc, dh * 512:(dh + 1) * 512],
                            start=(oc == 0), stop=(oc == 1))
                    nc.vector.tensor_copy(ysb[:, dh * 512:(dh + 1) * 512], yp[:])
                nc.sync.dma_start(out_d[scols, :], ysb[:])


# ---------------------------------------------------------------- host side
def _prep_inputs(x, wq, wk, wv, wo, freqs_cos, freqs_sin):
    """Shard + lay out host-side. Returns list of 8 in_maps."""
    bf = ml_dtypes.bfloat16
    # even/odd pair permutation within each head's 64 rows
    perm = np.concatenate([np.arange(0, HD, 2), np.arange(1, HD, 2)])
    cost = np.ascontiguousarray(freqs_cos.T).astype(np.float32)
    sint = np.ascontiguousarray(freqs_sin.T).astype(np.float32)
    tri = np.triu(np.ones((128, 128), np.float32)).astype(bf)  # [l, q]: l <= q

    in_maps = []
    for c in range(N_CORES):
        b, g = divmod(c, 4)
        xt = np.ascontiguousarray(x[b].T).astype(bf)
        wq_g = wq[g * GO:(g + 1) * GO].reshape(NH, HD, D)[:, perm, :].reshape(GO, D)
        wqt = np.ascontiguousarray(wq_g.T).astype(bf)
        wkt = np.ascontiguousarray(wk[g * HD:(g + 1) * HD][perm].T).astype(bf)
        wvt = np.ascontiguousarray(wv[g * HD:(g + 1) * HD].T).astype(bf)
        wot = np.ascontiguousarray(wo[:, g * GO:(g + 1) * GO].T).astype(bf)
        in_maps.append({
            "xt": xt, "wqt": wqt, "wkt": wkt, "wvt": wvt, "wot": wot,
            "cost": cost, "sint": sint, "tri": tri,
        })
    return in_maps


def get_nc():
    if "nc" not in _CACHE:
        _CACHE["nc"] = _build()
    return _CACHE["nc"]


def _ensure_ntff_hook():
    """The image's antenv lacks axon_hooks; inject an equivalent module so
    run_bass_kernel_spmd(trace=True) can capture NTFF profiles via the
    libaxon_pjrt.so C ABI (same shim trn_boot would register)."""
    import sys as _sys
    import types
    if "antenv.axon_hooks" in _sys.modules:
        return
    import contextlib
    import ctypes

    def _make_hook(so_path="/opt/axon/libaxon_pjrt.so"):
        try:
            lib = ctypes.CDLL(so_path)
        except OSError:
            return None
        if not hasattr(lib, "axon_start_nrt_profile"):
            return None
        lib.axon_start_nrt_profile.argtypes = [ctypes.POINTER(ctypes.c_int64),
                                               ctypes.c_size_t]
        lib.axon_start_nrt_profile.restype = ctypes.c_int64
        lib.axon_stop_nrt_profile.argtypes = [ctypes.c_char_p]
        lib.axon_stop_nrt_profile.restype = ctypes.c_int64

        @contextlib.contextmanager
        def _hook(output_dir, device_ids):
            import jax
            jax.devices()
            if device_ids:
                ids = (ctypes.c_int64 * len(device_ids))(*device_ids)
                rc = lib.axon_start_nrt_profile(ids, len(device_ids))
            else:
                rc = lib.axon_start_nrt_profile(None, 0)
            if rc != 0:
                raise RuntimeError(f"axon_start_nrt_profile rc={rc}")
            try:
                yield
            finally:
                n = lib.axon_stop_nrt_profile(str(output_dir).encode())
                print(f"profile: {n} file(s) -> {output_dir}", file=sys.stderr)

        return _hook

    hook = _make_hook()
    mod = types.ModuleType("antenv.axon_hooks")
    mod.get_axon_ntff_profile_hook = lambda: hook
    mod.set_axon_ntff_profile_hook = lambda h: None
    _sys.modules["antenv.axon_hooks"] = mod


def run(inputs, trace=False):
    from concourse.bass_utils import run_bass_kernel_spmd
    if trace:
        _ensure_ntff_hook()
    nc = get_nc()
    in_maps = _prep_inputs(**inputs)
    res = run_bass_kernel_spmd(nc, in_maps, core_ids=list(range(N_CORES)),
                               trace=trace)
    return res


def kernel(**inputs) -> np.ndarray:
    res = run(inputs)
    outs = [r["out"] for r in res.results]
    y = np.stack([outs[4 * b] + outs[4 * b + 1] + outs[4 * b + 2] + outs[4 * b + 3]
                  for b in range(B)])
    return y.astype(np.float32)


# revision 12
# speedup vs baseline: 1.1404x; 1.1404x over previous
"""Trainium2 Bass kernel for GQA attention (B=2, S=2048, D=1024, 16 q heads,
4 kv heads, head_dim 64, RoPE, causal).

Sharding: 8 cores = 2 (batch) x 4 (kv-head groups). Each core computes, for
its batch b and kv group g: the 4 query heads of group g + 1 kv head, plus the
partial output projection y_partial = attn_out_g @ wo[:, g_cols].T.  The host
unshard step sums the 4 partials per batch (the canonical all-reduce of
row-parallel TP, done on host since each core's output is already needed
host-side).

Device-side layout choices (all matmuls contract over the partition dim):
  - x is fed transposed (D on partitions) so QKV projections produce Q^T/K^T
    (head_dim on partitions, seq on free dim).
  - RoPE: wq/wk rows are permuted on host so lanes 0-31 are the "real" pair
    lanes and 32-63 the "imag" lanes; RoPE is then 2 full-width multiplies
    against replicated [c;s;c;s] tiles + 4 narrow combines on the DVE.
    (The permutation cancels in Q.K^T.)
  - Scores are computed as S^T (keys on partitions, queries on free):
    lhsT = K^T block, rhs = Q^T block.  Softmax needs no max-subtraction
    (|scores/8| <~ 3), so exp runs directly on the PSUM scores; the
    denominator is produced by an extra ones-row in the V stationary
    (out row 64 of the PV matmul = sum_l P^T[l, q]).
  - S blocks are paired into 2-bank PSUM tiles so each exp ACTIVATE covers
    (128, 1024) — halves ScalarE instruction + semaphore overhead.
  - Causal mask: matmuls are only emitted for the lower-triangle blocks; the
    128x128 diagonal blocks are masked multiplicatively (tri mask) after exp.
  - Normalization (divide by denominator, which lives along the free dim):
    reciprocal_approx_fast + gpsimd partition_broadcast + one DVE multiply.
"""

import sys

sys.path.insert(0, "/opt/trn_rl_repo")

from contextlib import ExitStack

import ml_dtypes
import numpy as np

import concourse.bass as bass
import concourse.mybir as mybir
import concourse.tile as tile
from concourse import bacc
from concourse.masks import make_identity

# ---------------------------------------------------------------- constants
B, S, D = 2, 2048, 1024
HD = 64
HALF = HD // 2
HKV = 4          # kv heads total
NH = 4           # q heads per core (= NREP)
KVD = HKV * HD   # 256
GO = NH * HD     # 256 output features per group
N_CORES = 8

SB = 512         # q superblock (matmul free dim)
NQS = S // SB    # 4 q superblocks
NLB = S // 128   # 16 key blocks of 128
KCH = D // 128   # 8 contraction chunks for projections

F32 = mybir.dt.float32
BF16 = mybir.dt.bfloat16
SCALE = 1.0 / 8.0  # 1/sqrt(64)
EXP = mybir.ActivationFunctionType.Exp

_CACHE = {}
DEBUG_DUMPS = False  # set True (before get_nc) to add intermediate outputs


# ---------------------------------------------------------------- builder
def _build():
    nc = bacc.Bacc("TRN2", target_bir_lowering=False, debug=False,
                   enable_asserts=False, num_devices=N_CORES)

    xt_d = nc.dram_tensor("xt", [D, S], BF16, kind="ExternalInput").ap()
    wqt_d = nc.dram_tensor("wqt", [D, GO], BF16, kind="ExternalInput").ap()
    wkt_d = nc.dram_tensor("wkt", [D, HD], BF16, kind="ExternalInput").ap()
    wvt_d = nc.dram_tensor("wvt", [D, HD], BF16, kind="ExternalInput").ap()
    wot_d = nc.dram_tensor("wot", [GO, D], BF16, kind="ExternalInput").ap()
    cost_d = nc.dram_tensor("cost", [HALF, S], F32, kind="ExternalInput").ap()
    sint_d = nc.dram_tensor("sint", [HALF, S], F32, kind="ExternalInput").ap()
    tri_d = nc.dram_tensor("tri", [128, 128], BF16, kind="ExternalInput").ap()
    out_d = nc.dram_tensor("out", [S, D], F32, kind="ExternalOutput").ap()
    dbg = {}
    if DEBUG_DUMPS:
        dbg["qT"] = nc.dram_tensor("dbg_qT", [128, 2, S], BF16,
                                   kind="ExternalOutput").ap()
        dbg["kT"] = nc.dram_tensor("dbg_kT", [128, S], BF16,
                                   kind="ExternalOutput").ap()
        dbg["v"] = nc.dram_tensor("dbg_v", [128, NLB, HD + 1], BF16,
                                  kind="ExternalOutput").ap()
        dbg["att"] = nc.dram_tensor("dbg_att", [128, 2, S], BF16,
                                    kind="ExternalOutput").ap()

    with ExitStack() as ctx:
        tc = ctx.enter_context(tile.TileContext(nc))
        _emit(nc, tc, ctx, xt_d, wqt_d, wkt_d, wvt_d, wot_d, cost_d, sint_d,
              tri_d, out_d, dbg)

    nc.compile()
    return nc


def _emit(nc, tc, ctx, xt_d, wqt_d, wkt_d, wvt_d, wot_d, cost_d, sint_d,
          tri_d, out_d, dbg={}):
    perm = ctx.enter_context(tc.tile_pool(name="perm", bufs=1))
    pexp = ctx.enter_context(tc.tile_pool(name="pexp", bufs=4))
    ptmp = ctx.enter_context(tc.tile_pool(name="ptmp", bufs=3))
    pout = ctx.enter_context(tc.tile_pool(name="pout", bufs=3))
    pp_mm = ctx.enter_context(tc.tile_pool(name="ppmm", bufs=2, space="PSUM"))

    # ---------------- persistent SBUF tensors
    xt_sb = perm.tile([128, KCH, S], BF16, tag="xt")
    wqt_sb = perm.tile([128, KCH, GO], BF16, tag="wqt")
    wkt_sb = perm.tile([128, KCH, HD], BF16, tag="wkt")
    wvt_sb = perm.tile([128, KCH, HD], BF16, tag="wvt")
    wot_sb = perm.tile([128, 2, D], BF16, tag="wot")
    cos4_sb = perm.tile([128, S], F32, tag="cos4")       # cos replicated 4x
    sin4_sb = perm.tile([128, S], F32, tag="sin4")       # sin replicated 4x
    tri_sb = perm.tile([128, 128], BF16, tag="tri")
    ident = perm.tile([64, 64], BF16, tag="ident")
    qT_sb = perm.tile([128, 2, S], BF16, tag="qT")       # [hd|hd, mi, s]
    kT_sb = perm.tile([128, S], BF16, tag="kT")          # rows 64-127 = dup
    v_sb = perm.tile([128, NLB, HD + 1], BF16, tag="v")  # [l, lb, hd|1]
    att_sb = perm.tile([128, 2, S], BF16, tag="att")     # [o%128, o//128, s]

    # ---------------- input DMAs
    nc.sync.dma_start(wqt_sb[:], wqt_d.rearrange("(kc p) m -> p kc m", p=128))
    nc.sync.dma_start(wkt_sb[:], wkt_d.rearrange("(kc p) m -> p kc m", p=128))
    nc.sync.dma_start(wvt_sb[:], wvt_d.rearrange("(kc p) m -> p kc m", p=128))
    nc.sync.dma_start(wot_sb[:], wot_d.rearrange("(oc p) d -> p oc d", p=128))
    for q in range(4):  # replicate cos/sin across all four 32-row groups
        nc.sync.dma_start(cos4_sb[q * 32:(q + 1) * 32, :], cost_d)
        nc.sync.dma_start(sin4_sb[q * 32:(q + 1) * 32, :], sint_d)
    nc.sync.dma_start(tri_sb[:], tri_d)
    xt_r = xt_d.rearrange("(kc p) s -> p kc s", p=128)
    for si in range(NQS):
        nc.sync.dma_start(xt_sb[:, :, si * SB:(si + 1) * SB],
                          xt_r[:, :, si * SB:(si + 1) * SB])
    make_identity(nc, ident[:])

    # ---------------- helper: RoPE on a psum projection tile
    # ps rows per 64-row head block: [real(32); imag(32)].  m0 = ps*cos in
    # SBUF, m1 = ps*sin in PSUM; each combine then mixes one SBUF operand
    # with one PSUM operand so the cross-partition pairing stays legal
    # (walrus requires all SBUF APs of a DVE op on identical partitions).
    def rope(ps, nrow, cols, dst, pool_m1):
        m0 = ptmp.tile([128, SB], BF16, tag="ropem0", name="m0")[0:nrow]
        m1 = pool_m1.tile([128, SB], F32, tag="ropem1", name="m1")[0:nrow]
        nc.vector.tensor_mul(m0[:], ps, cos4_sb[0:nrow, cols])
        nc.vector.tensor_mul(m1[:], ps, sin4_sb[0:nrow, cols])
        for b0 in range(0, nrow, 64):
            # out_r = r*c - i*s ; out_i = r*s + i*c
            nc.vector.tensor_sub(dst[b0:b0 + 32], m0[b0:b0 + 32, :],
                                 m1[b0 + 32:b0 + 64, :])
            nc.vector.tensor_add(dst[b0 + 32:b0 + 64], m1[b0:b0 + 32, :],
                                 m0[b0 + 32:b0 + 64, :])

    # ---------------- projections (stage B)
    nc.vector.memset(v_sb[:, :, HD:HD + 1], 1.0)  # ones column -> denom
    with tc.tile_pool(name="pptr", bufs=2, space="PSUM") as pp_tr:
        # Q projection + RoPE
        for mi in range(2):       # two 128-row chunks = heads (2mi, 2mi+1)
            for si in range(NQS):
                cols = slice(si * SB, (si + 1) * SB)
                ps = pp_mm.tile([128, SB], F32, tag="mm")
                for kc in range(KCH):
                    nc.tensor.matmul(
                        ps[:], wqt_sb[:, kc, mi * 128:(mi + 1) * 128],
                        xt_sb[:, kc, cols], start=(kc == 0), stop=(kc == KCH - 1))
                rope(ps[:], 128, cols, qT_sb[:, mi, cols], pp_tr)

        # K projection + RoPE
        for si in range(NQS):
            cols = slice(si * SB, (si + 1) * SB)
            psf = pp_mm.tile([128, SB], F32, tag="mm")
            ps = psf[0:64]
            for kc in range(KCH):
                nc.tensor.matmul(ps[:], wkt_sb[:, kc, :], xt_sb[:, kc, cols],
                                 start=(kc == 0), stop=(kc == KCH - 1))
            rope(ps[:], 64, cols, kT_sb[0:64, cols], pp_tr)
        # duplicate K^T to partitions 64-127 for the odd-head row-tiled matmuls
        nc.sync.dma_start(kT_sb[64:128, :], kT_sb[0:64, :])

        # V projection (V^T), then transpose to natural V
        for si in range(NQS):
            cols = slice(si * SB, (si + 1) * SB)
            psf = pp_mm.tile([128, SB], F32, tag="mm")
            ps = psf[0:64]
            for kc in range(KCH):
                nc.tensor.matmul(ps[:], wvt_sb[:, kc, :], xt_sb[:, kc, cols],
                                 start=(kc == 0), stop=(kc == KCH - 1))
            vt = ptmp.tile([64, SB], BF16, tag="vtstage")
            nc.vector.tensor_copy(vt[:], ps[:])
            for j in range(SB // 128):
                lb = si * (SB // 128) + j
                pt = pp_tr.tile([128, 64], BF16, tag="tr")
                nc.tensor.transpose(pt[:], vt[:, j * 128:(j + 1) * 128], ident[:])
                nc.vector.tensor_copy(v_sb[:, lb, 0:HD], pt[:])

    # ---------------- attention + output projection, per q superblock
    pp_sb = ctx.enter_context(tc.tile_pool(name="ppsb", bufs=2, space="PSUM"))
    pp_pv = ctx.enter_context(tc.tile_pool(name="pppv", bufs=2, space="PSUM"))

    def wo_chunk(si):
        """output projection for one 128-row q chunk"""
        scols = slice(si * 128, (si + 1) * 128)
        ysb = pout.tile([128, D], F32, tag="ysb", name="ysb")
        for dh in range(2):
            yp = pp_mm.tile([128, 512], F32, tag="mm", name="yp")
            for oc in range(2):
                nc.tensor.matmul(
                    yp[:], att_sb[:, oc, scols],
                    wot_sb[:, oc, dh * 512:(dh + 1) * 512],
                    start=(oc == 0), stop=(oc == 1))
            nc.vector.tensor_copy(ysb[:, dh * 512:(dh + 1) * 512], yp[:])
        nc.sync.dma_start(out_d[scols, :], ysb[:])

    for qs in range(NQS):
        qcols = slice(qs * SB, (qs + 1) * SB)
        nlb = 4 * qs + 4          # key blocks needed (block-causal)
        for mi in range(2):       # head pair (2mi, 2mi+1) at partitions 0/64
            po0 = pp_pv.tile([HD + 1, SB], F32, tag="pv", name="po0")
            po1 = pp_pv.tile([HD + 1, SB], F32, tag="pv", name="po1")
            pos = (po0, po1)
            for lb in range(nlb):
                j = lb - 4 * qs   # >=0 on the diagonal superblock
                kcols = slice(lb * 128, (lb + 1) * 128)
                sp = pp_sb.tile([128, 2, SB], F32, tag="sbig", name="sp")
                # the two matmuls run concurrently (row groups 0-1 / 2-3)
                nc.tensor.matmul(sp[:, 0, :], kT_sb[0:64, kcols],
                                 qT_sb[0:64, mi, qcols], start=True, stop=True)
                nc.tensor.matmul(sp[:, 1, :], kT_sb[64:128, kcols],
                                 qT_sb[64:128, mi, qcols], start=True, stop=True)
                pe = pexp.tile([128, 2, SB], BF16, tag="pexp", name="pe")
                nc.scalar.activation(pe[:, 0, :], sp[:, 0, :], EXP, scale=SCALE)
                nc.scalar.activation(pe[:, 1, :], sp[:, 1, :], EXP, scale=SCALE)
                if j >= 0:
                    dcols = slice(j * 128, (j + 1) * 128)
                    nc.vector.tensor_mul(pe[:, 0, dcols], pe[:, 0, dcols],
                                         tri_sb[:])
                    nc.vector.tensor_mul(pe[:, 1, dcols], pe[:, 1, dcols],
                                         tri_sb[:])
                vcols = slice(max(j, 0) * 128, SB)
                for i in range(2):
                    nc.tensor.matmul(pos[i][:, vcols], v_sb[:, lb, :],
                                     pe[:, i, vcols],
                                     start=(lb == 0), stop=(lb == nlb - 1))
            # normalize: att = po[0:64] * (1/denom) broadcast over partitions
            for i in range(2):
                h = 2 * mi + i
                b0 = i * 64
                rc = ptmp.tile([1, SB], F32, tag="recip", name="rc")
                nc.vector.reciprocal(rc[:], pos[i][HD:HD + 1, :])
                rb = ptmp.tile([128, SB], F32, tag="recipb", name="rb")
                nc.gpsimd.partition_broadcast(rb[:], rc[:])
                nc.vector.tensor_mul(
                    att_sb[b0:b0 + 64, mi, qcols], pos[i][0:HD, :],
                    rb[b0:b0 + 64, :])
            # interleave previous superblock's output projection here so the
            # TensorE has dense work while ScalarE chews the exp backlog
            if qs > 0:
                wo_chunk((qs - 1) * 4 + 2 * mi)
                wo_chunk((qs - 1) * 4 + 2 * mi + 1)
    for sj in range(4):
        wo_chunk(3 * 4 + sj)
    if dbg:
        nc.sync.dma_start(dbg["qT"], qT_sb[:])
        nc.sync.dma_start(dbg["kT"], kT_sb[:])
        nc.sync.dma_start(dbg["v"], v_sb[:])
        nc.sync.dma_start(dbg["att"], att_sb[:])


# ---------------------------------------------------------------- host side
def _prep_inputs(x, wq, wk, wv, wo, freqs_cos, freqs_sin):
    """Shard + lay out host-side. Returns list of 8 in_maps."""
    bf = ml_dtypes.bfloat16
    # even/odd pair permutation within each head's 64 rows
    perm = np.concatenate([np.arange(0, HD, 2), np.arange(1, HD, 2)])
    cost = np.ascontiguousarray(freqs_cos.T).astype(np.float32)
    sint = np.ascontiguousarray(freqs_sin.T).astype(np.float32)
    tri = np.triu(np.ones((128, 128), np.float32)).astype(bf)  # [l, q]: l <= q

    in_maps = []
    for c in range(N_CORES):
        b, g = divmod(c, 4)
        xt = np.ascontiguousarray(x[b].T).astype(bf)
        wq_g = wq[g * GO:(g + 1) * GO].reshape(NH, HD, D)[:, perm, :].reshape(GO, D)
        wqt = np.ascontiguousarray(wq_g.T).astype(bf)
        wkt = np.ascontiguousarray(wk[g * HD:(g + 1) * HD][perm].T).astype(bf)
        wvt = np.ascontiguousarray(wv[g * HD:(g + 1) * HD].T).astype(bf)
        wot = np.ascontiguousarray(wo[:, g * GO:(g + 1) * GO].T).astype(bf)
        in_maps.append({
            "xt": xt, "wqt": wqt, "wkt": wkt, "wvt": wvt, "wot": wot,
            "cost": cost, "sint": sint, "tri": tri,
        })
    return in_maps


def get_nc():
    if "nc" not in _CACHE:
        _CACHE["nc"] = _build()
    return _CACHE["nc"]


def _ensure_ntff_hook():
    """The image's antenv lacks axon_hooks; inject an equivalent module so
    run_bass_kernel_spmd(trace=True) can capture NTFF profiles via the
    libaxon_pjrt.so C ABI (same shim trn_boot would register)."""
    import sys as _sys
    import types
    if "antenv.axon_hooks" in _sys.modules:
        return
    import contextlib
    import ctypes

    def _make_hook(so_path="/opt/axon/libaxon_pjrt.so"):
        try:
            lib = ctypes.CDLL(so_path)
        except OSError:
            return None
        if not hasattr(lib, "axon_start_nrt_profile"):
            return None
        lib.axon_start_nrt_profile.argtypes = [ctypes.POINTER(ctypes.c_int64),
                                               ctypes.c_size_t]
        lib.axon_start_nrt_profile.restype = ctypes.c_int64
        lib.axon_stop_nrt_profile.argtypes = [ctypes.c_char_p]
        lib.axon_stop_nrt_profile.restype = ctypes.c_int64

        @contextlib.contextmanager
        def _hook(output_dir, device_ids):
            import jax
            jax.devices()
            if device_ids:
                ids = (ctypes.c_int64 * len(device_ids))(*device_ids)
                rc = lib.axon_start_nrt_profile(ids, len(device_ids))
            else:
                rc = lib.axon_start_nrt_profile(None, 0)
            if rc != 0:
                raise RuntimeError(f"axon_start_nrt_profile rc={rc}")
            try:
                yield
            finally:
                n = lib.axon_stop_nrt_profile(str(output_dir).encode())
                print(f"profile: {n} file(s) -> {output_dir}", file=sys.stderr)

        return _hook

    hook = _make_hook()
    mod = types.ModuleType("antenv.axon_hooks")
    mod.get_axon_ntff_profile_hook = lambda: hook
    mod.set_axon_ntff_profile_hook = lambda h: None
    _sys.modules["antenv.axon_hooks"] = mod


def run(inputs, trace=False):
    from concourse.bass_utils import run_bass_kernel_spmd
    if trace:
        _ensure_ntff_hook()
    nc = get_nc()
    in_maps = _prep_inputs(**inputs)
    res = run_bass_kernel_spmd(nc, in_maps, core_ids=list(range(N_CORES)),
                               trace=trace)
    return res


def kernel(**inputs) -> np.ndarray:
    res = run(inputs)
    outs = [r["out"] for r in res.results]
    y = np.stack([outs[4 * b] + outs[4 * b + 1] + outs[4 * b + 2] + outs[4 * b + 3]
                  for b in range(B)])
    return y.astype(np.float32)


# revision 14
# speedup vs baseline: 1.2222x; 1.0718x over previous
"""Trainium2 Bass kernel for GQA attention (B=2, S=2048, D=1024, 16 q heads,
4 kv heads, head_dim 64, RoPE, causal).

Sharding: 8 cores = 2 (batch) x 4 (kv-head groups). Each core computes, for
its batch b and kv group g: the 4 query heads of group g + 1 kv head, plus the
partial output projection y_partial = attn_out_g @ wo[:, g_cols].T.  The host
unshard step sums the 4 partials per batch (the canonical all-reduce of
row-parallel TP, done on host since each core's output is already needed
host-side).

Device-side layout choices (all matmuls contract over the partition dim):
  - x is fed transposed (D on partitions) so QKV projections produce Q^T/K^T
    (head_dim on partitions, seq on free dim).
  - RoPE: wq/wk rows are permuted on host so lanes 0-31 are the "real" pair
    lanes and 32-63 the "imag" lanes; RoPE is then 2 full-width multiplies
    against replicated [c;s;c;s] tiles + 4 narrow combines on the DVE.
    (The permutation cancels in Q.K^T.)
  - Scores are computed as S^T (keys on partitions, queries on free):
    lhsT = K^T block, rhs = Q^T block.  Softmax needs no max-subtraction
    (|scores/8| <~ 3), so exp runs directly on the PSUM scores; the
    denominator is produced by an extra ones-row in the V stationary
    (out row 64 of the PV matmul = sum_l P^T[l, q]).
  - S blocks are paired into 2-bank PSUM tiles so each exp ACTIVATE covers
    (128, 1024) — halves ScalarE instruction + semaphore overhead.
  - Causal mask: matmuls are only emitted for the lower-triangle blocks; the
    128x128 diagonal blocks are masked multiplicatively (tri mask) after exp.
  - Normalization (divide by denominator, which lives along the free dim):
    reciprocal_approx_fast + gpsimd partition_broadcast + one DVE multiply.
"""

import sys

sys.path.insert(0, "/opt/trn_rl_repo")

from contextlib import ExitStack

import ml_dtypes
import numpy as np

import concourse.bass as bass
import concourse.mybir as mybir
import concourse.tile as tile
from concourse import bacc
from concourse.masks import make_identity

# ---------------------------------------------------------------- constants
B, S, D = 2, 2048, 1024
HD = 64
HALF = HD // 2
HKV = 4          # kv heads total
NH = 4           # q heads per core (= NREP)
KVD = HKV * HD   # 256
GO = NH * HD     # 256 output features per group
N_CORES = 8

SB = 512         # q superblock (matmul free dim)
NQS = S // SB    # 4 q superblocks
NLB = S // 128   # 16 key blocks of 128
KCH = D // 128   # 8 contraction chunks for projections

F32 = mybir.dt.float32
BF16 = mybir.dt.bfloat16
SCALE = 1.0 / 8.0  # 1/sqrt(64)
EXP = mybir.ActivationFunctionType.Exp

_CACHE = {}
DEBUG_DUMPS = False  # set True (before get_nc) to add intermediate outputs


# ---------------------------------------------------------------- builder
def _build():
    nc = bacc.Bacc("TRN2", target_bir_lowering=False, debug=False,
                   enable_asserts=False, num_devices=N_CORES)

    xt_d = nc.dram_tensor("xt", [D, S], BF16, kind="ExternalInput").ap()
    wqt_d = nc.dram_tensor("wqt", [D, GO], BF16, kind="ExternalInput").ap()
    wkt_d = nc.dram_tensor("wkt", [D, HD], BF16, kind="ExternalInput").ap()
    wvt_d = nc.dram_tensor("wvt", [D, HD], BF16, kind="ExternalInput").ap()
    wot_d = nc.dram_tensor("wot", [GO, D], BF16, kind="ExternalInput").ap()
    cost_d = nc.dram_tensor("cost", [HALF, S], F32, kind="ExternalInput").ap()
    sint_d = nc.dram_tensor("sint", [HALF, S], F32, kind="ExternalInput").ap()
    tri_d = nc.dram_tensor("tri", [128, 128], BF16, kind="ExternalInput").ap()
    out_d = nc.dram_tensor("out", [S, D], F32, kind="ExternalOutput").ap()
    dbg = {}
    if DEBUG_DUMPS:
        dbg["qT"] = nc.dram_tensor("dbg_qT", [128, 2, S], BF16,
                                   kind="ExternalOutput").ap()
        dbg["kT"] = nc.dram_tensor("dbg_kT", [128, S], BF16,
                                   kind="ExternalOutput").ap()
        dbg["v"] = nc.dram_tensor("dbg_v", [128, NLB, HD + 1], BF16,
                                  kind="ExternalOutput").ap()
        dbg["att"] = nc.dram_tensor("dbg_att", [128, 2, S], BF16,
                                    kind="ExternalOutput").ap()

    with ExitStack() as ctx:
        tc = ctx.enter_context(tile.TileContext(nc))
        _emit(nc, tc, ctx, xt_d, wqt_d, wkt_d, wvt_d, wot_d, cost_d, sint_d,
              tri_d, out_d, dbg)

    nc.compile()
    return nc


def _emit(nc, tc, ctx, xt_d, wqt_d, wkt_d, wvt_d, wot_d, cost_d, sint_d,
          tri_d, out_d, dbg={}):
    perm = ctx.enter_context(tc.tile_pool(name="perm", bufs=1))
    pexp = ctx.enter_context(tc.tile_pool(name="pexp", bufs=4))
    ptmp = ctx.enter_context(tc.tile_pool(name="ptmp", bufs=3))
    pout = ctx.enter_context(tc.tile_pool(name="pout", bufs=3))
    pp_mm = ctx.enter_context(tc.tile_pool(name="ppmm", bufs=2, space="PSUM"))

    # ---------------- persistent SBUF tensors
    xt_sb = perm.tile([128, KCH, S], BF16, tag="xt")
    wqt_sb = perm.tile([128, KCH, GO], BF16, tag="wqt")
    wkt_sb = perm.tile([128, KCH, HD], BF16, tag="wkt")
    wvt_sb = perm.tile([128, KCH, HD], BF16, tag="wvt")
    wot_sb = perm.tile([128, 2, D], BF16, tag="wot")
    cos4_sb = perm.tile([128, S], F32, tag="cos4")       # cos replicated 4x
    sin4_sb = perm.tile([128, S], F32, tag="sin4")       # sin replicated 4x
    tri_sb = perm.tile([128, 128], BF16, tag="tri")
    ident = perm.tile([64, 64], BF16, tag="ident")
    qT_sb = perm.tile([128, 2, S], BF16, tag="qT")       # [hd|hd, mi, s]
    kT_sb = perm.tile([128, S], BF16, tag="kT")          # rows 64-127 = dup
    v_sb = perm.tile([128, NLB, HD + 1], BF16, tag="v")  # [l, lb, hd|1]
    att_sb = perm.tile([128, 2, S], BF16, tag="att")     # [o%128, o//128, s]

    # ---------------- input DMAs
    nc.sync.dma_start(wqt_sb[:], wqt_d.rearrange("(kc p) m -> p kc m", p=128))
    nc.sync.dma_start(wkt_sb[:], wkt_d.rearrange("(kc p) m -> p kc m", p=128))
    nc.sync.dma_start(wvt_sb[:], wvt_d.rearrange("(kc p) m -> p kc m", p=128))
    nc.sync.dma_start(wot_sb[:], wot_d.rearrange("(oc p) d -> p oc d", p=128))
    for q in range(4):  # replicate cos/sin across all four 32-row groups
        nc.sync.dma_start(cos4_sb[q * 32:(q + 1) * 32, :], cost_d)
        nc.sync.dma_start(sin4_sb[q * 32:(q + 1) * 32, :], sint_d)
    nc.sync.dma_start(tri_sb[:], tri_d)
    xt_r = xt_d.rearrange("(kc p) s -> p kc s", p=128)
    for si in range(NQS):
        nc.sync.dma_start(xt_sb[:, :, si * SB:(si + 1) * SB],
                          xt_r[:, :, si * SB:(si + 1) * SB])
    make_identity(nc, ident[:])

    # ---------------- helper: RoPE on a psum projection tile
    # ps rows per 64-row head block: [real(32); imag(32)].  m0 = ps*cos in
    # SBUF, m1 = ps*sin in PSUM; each combine then mixes one SBUF operand
    # with one PSUM operand so the cross-partition pairing stays legal
    # (walrus requires all SBUF APs of a DVE op on identical partitions).
    def rope(ps, nrow, cols, dst, pool_m1):
        m0 = ptmp.tile([128, SB], BF16, tag="ropem0", name="m0")[0:nrow]
        m1 = pool_m1.tile([128, SB], F32, tag="ropem1", name="m1")[0:nrow]
        nc.vector.tensor_mul(m0[:], ps, cos4_sb[0:nrow, cols])
        nc.vector.tensor_mul(m1[:], ps, sin4_sb[0:nrow, cols])
        for b0 in range(0, nrow, 64):
            # out_r = r*c - i*s ; out_i = r*s + i*c
            nc.vector.tensor_sub(dst[b0:b0 + 32], m0[b0:b0 + 32, :],
                                 m1[b0 + 32:b0 + 64, :])
            nc.vector.tensor_add(dst[b0 + 32:b0 + 64], m1[b0:b0 + 32, :],
                                 m0[b0 + 32:b0 + 64, :])

    # ---------------- projections (stage B)
    nc.vector.memset(v_sb[:, :, HD:HD + 1], 1.0)  # ones column -> denom
    with tc.tile_pool(name="pptr", bufs=2, space="PSUM") as pp_tr:
        # Q projection + RoPE
        for mi in range(2):       # two 128-row chunks = heads (2mi, 2mi+1)
            for si in range(NQS):
                cols = slice(si * SB, (si + 1) * SB)
                ps = pp_mm.tile([128, SB], F32, tag="mm")
                for kc in range(KCH):
                    nc.tensor.matmul(
                        ps[:], wqt_sb[:, kc, mi * 128:(mi + 1) * 128],
                        xt_sb[:, kc, cols], start=(kc == 0), stop=(kc == KCH - 1))
                rope(ps[:], 128, cols, qT_sb[:, mi, cols], pp_tr)

        # K projection + RoPE
        for si in range(NQS):
            cols = slice(si * SB, (si + 1) * SB)
            psf = pp_mm.tile([128, SB], F32, tag="mm")
            ps = psf[0:64]
            for kc in range(KCH):
                nc.tensor.matmul(ps[:], wkt_sb[:, kc, :], xt_sb[:, kc, cols],
                                 start=(kc == 0), stop=(kc == KCH - 1))
            rope(ps[:], 64, cols, kT_sb[0:64, cols], pp_tr)
        # duplicate K^T to partitions 64-127 for the odd-head row-tiled matmuls
        nc.sync.dma_start(kT_sb[64:128, :], kT_sb[0:64, :])

        # V projection (V^T), then transpose to natural V
        for si in range(NQS):
            cols = slice(si * SB, (si + 1) * SB)
            psf = pp_mm.tile([128, SB], F32, tag="mm")
            ps = psf[0:64]
            for kc in range(KCH):
                nc.tensor.matmul(ps[:], wvt_sb[:, kc, :], xt_sb[:, kc, cols],
                                 start=(kc == 0), stop=(kc == KCH - 1))
            vt = ptmp.tile([64, SB], BF16, tag="vtstage")
            nc.vector.tensor_copy(vt[:], ps[:])
            for j in range(SB // 128):
                lb = si * (SB // 128) + j
                pt = pp_tr.tile([128, 64], BF16, tag="tr")
                nc.tensor.transpose(pt[:], vt[:, j * 128:(j + 1) * 128], ident[:])
                nc.vector.tensor_copy(v_sb[:, lb, 0:HD], pt[:])

    # ---------------- attention + output projection, per q superblock
    pp_sb = ctx.enter_context(tc.tile_pool(name="ppsb", bufs=2, space="PSUM"))
    pp_pv = ctx.enter_context(tc.tile_pool(name="pppv", bufs=2, space="PSUM"))

    def wo_chunk(si):
        """output projection for one 128-row q chunk"""
        scols = slice(si * 128, (si + 1) * 128)
        ysb = pout.tile([128, D], F32, tag="ysb", name="ysb")
        for dh in range(2):
            yp = pp_mm.tile([128, 512], F32, tag="mm", name="yp")
            for oc in range(2):
                nc.tensor.matmul(
                    yp[:], att_sb[:, oc, scols],
                    wot_sb[:, oc, dh * 512:(dh + 1) * 512],
                    start=(oc == 0), stop=(oc == 1))
            nc.vector.tensor_copy(ysb[:, dh * 512:(dh + 1) * 512], yp[:])
        nc.sync.dma_start(out_d[scols, :], ysb[:])

    for qs in range(NQS):
        qcols = slice(qs * SB, (qs + 1) * SB)
        nlb = 4 * qs + 4          # key blocks needed (block-causal)
        for mi in range(2):       # head pair (2mi, 2mi+1) at partitions 0/64
            po0 = pp_pv.tile([HD + 1, SB], F32, tag="pv", name="po0")
            po1 = pp_pv.tile([HD + 1, SB], F32, tag="pv", name="po1")
            pos = (po0, po1)
            for lb in range(nlb):
                j = lb - 4 * qs   # >=0 on the diagonal superblock
                kcols = slice(lb * 128, (lb + 1) * 128)
                sp = pp_sb.tile([128, 2, SB], F32, tag="sbig", name="sp")
                # the two matmuls run concurrently (row groups 0-1 / 2-3)
                nc.tensor.matmul(sp[:, 0, :], kT_sb[0:64, kcols],
                                 qT_sb[0:64, mi, qcols], start=True, stop=True)
                nc.tensor.matmul(sp[:, 1, :], kT_sb[64:128, kcols],
                                 qT_sb[64:128, mi, qcols], start=True, stop=True)
                pe = pexp.tile([128, 2, SB], BF16, tag="pexp", name="pe")
                nc.scalar.activation(pe[:], sp[:], EXP, scale=SCALE)
                if j >= 0:
                    dcols = slice(j * 128, (j + 1) * 128)
                    nc.vector.tensor_mul(pe[:, 0, dcols], pe[:, 0, dcols],
                                         tri_sb[:])
                    nc.vector.tensor_mul(pe[:, 1, dcols], pe[:, 1, dcols],
                                         tri_sb[:])
                vcols = slice(max(j, 0) * 128, SB)
                for i in range(2):
                    nc.tensor.matmul(pos[i][:, vcols], v_sb[:, lb, :],
                                     pe[:, i, vcols],
                                     start=(lb == 0), stop=(lb == nlb - 1))
            # normalize: att = po[0:64] * (1/denom) broadcast over partitions
            for i in range(2):
                h = 2 * mi + i
                b0 = i * 64
                # 1/d = exp(-ln d) on ScalarE: avoids the 3.3us DVE
                # reciprocal (whose PE stall re-throttles HAM every head)
                rln = ptmp.tile([1, SB], F32, tag="recipln", name="rln")
                nc.scalar.activation(rln[:], pos[i][HD:HD + 1, :],
                                     mybir.ActivationFunctionType.Ln)
                rc = ptmp.tile([1, SB], F32, tag="recip", name="rc")
                nc.scalar.activation(rc[:], rln[:], EXP, scale=-1.0)
                rb = ptmp.tile([128, SB], F32, tag="recipb", name="rb")
                nc.gpsimd.partition_broadcast(rb[:], rc[:])
                nc.vector.tensor_mul(
                    att_sb[b0:b0 + 64, mi, qcols], pos[i][0:HD, :],
                    rb[b0:b0 + 64, :])
            # interleave previous superblock's output projection here so the
            # TensorE has dense work while ScalarE chews the exp backlog
            if qs > 0:
                wo_chunk((qs - 1) * 4 + 2 * mi)
                wo_chunk((qs - 1) * 4 + 2 * mi + 1)
    for sj in range(4):
        wo_chunk(3 * 4 + sj)
    if dbg:
        nc.sync.dma_start(dbg["qT"], qT_sb[:])
        nc.sync.dma_start(dbg["kT"], kT_sb[:])
        nc.sync.dma_start(dbg["v"], v_sb[:])
        nc.sync.dma_start(dbg["att"], att_sb[:])


# ---------------------------------------------------------------- host side
def _prep_inputs(x, wq, wk, wv, wo, freqs_cos, freqs_sin):
    """Shard + lay out host-side. Returns list of 8 in_maps."""
    bf = ml_dtypes.bfloat16
    # even/odd pair permutation within each head's 64 rows
    perm = np.concatenate([np.arange(0, HD, 2), np.arange(1, HD, 2)])
    cost = np.ascontiguousarray(freqs_cos.T).astype(np.float32)
    sint = np.ascontiguousarray(freqs_sin.T).astype(np.float32)
    tri = np.triu(np.ones((128, 128), np.float32)).astype(bf)  # [l, q]: l <= q

    in_maps = []
    for c in range(N_CORES):
        b, g = divmod(c, 4)
        xt = np.ascontiguousarray(x[b].T).astype(bf)
        wq_g = wq[g * GO:(g + 1) * GO].reshape(NH, HD, D)[:, perm, :].reshape(GO, D)
        wqt = np.ascontiguousarray(wq_g.T).astype(bf)
        wkt = np.ascontiguousarray(wk[g * HD:(g + 1) * HD][perm].T).astype(bf)
        wvt = np.ascontiguousarray(wv[g * HD:(g + 1) * HD].T).astype(bf)
        wot = np.ascontiguousarray(wo[:, g * GO:(g + 1) * GO].T).astype(bf)
        in_maps.append({
            "xt": xt, "wqt": wqt, "wkt": wkt, "wvt": wvt, "wot": wot,
            "cost": cost, "sint": sint, "tri": tri,
        })
    return in_maps


def get_nc():
    if "nc" not in _CACHE:
        _CACHE["nc"] = _build()
    return _CACHE["nc"]


def _ensure_ntff_hook():
    """The image's antenv lacks axon_hooks; inject an equivalent module so
    run_bass_kernel_spmd(trace=True) can capture NTFF profiles via the
    libaxon_pjrt.so C ABI (same shim trn_boot would register)."""
    import sys as _sys
    import types
    if "antenv.axon_hooks" in _sys.modules:
        return
    import contextlib
    import ctypes

    def _make_hook(so_path="/opt/axon/libaxon_pjrt.so"):
        try:
            lib = ctypes.CDLL(so_path)
        except OSError:
            return None
        if not hasattr(lib, "axon_start_nrt_profile"):
            return None
        lib.axon_start_nrt_profile.argtypes = [ctypes.POINTER(ctypes.c_int64),
                                               ctypes.c_size_t]
        lib.axon_start_nrt_profile.restype = ctypes.c_int64
        lib.axon_stop_nrt_profile.argtypes = [ctypes.c_char_p]
        lib.axon_stop_nrt_profile.restype = ctypes.c_int64

        @contextlib.contextmanager
        def _hook(output_dir, device_ids):
            import jax
            jax.devices()
            if device_ids:
                ids = (ctypes.c_int64 * len(device_ids))(*device_ids)
                rc = lib.axon_start_nrt_profile(ids, len(device_ids))
            else:
                rc = lib.axon_start_nrt_profile(None, 0)
            if rc != 0:
                raise RuntimeError(f"axon_start_nrt_profile rc={rc}")
            try:
                yield
            finally:
                n = lib.axon_stop_nrt_profile(str(output_dir).encode())
                print(f"profile: {n} file(s) -> {output_dir}", file=sys.stderr)

        return _hook

    hook = _make_hook()
    mod = types.ModuleType("antenv.axon_hooks")
    mod.get_axon_ntff_profile_hook = lambda: hook
    mod.set_axon_ntff_profile_hook = lambda h: None
    _sys.modules["antenv.axon_hooks"] = mod


def run(inputs, trace=False):
    from concourse.bass_utils import run_bass_kernel_spmd
    if trace:
        _ensure_ntff_hook()
    nc = get_nc()
    in_maps = _prep_inputs(**inputs)
    res = run_bass_kernel_spmd(nc, in_maps, core_ids=list(range(N_CORES)),
                               trace=trace)
    return res


def kernel(**inputs) -> np.ndarray:
    res = run(inputs)
    outs = [r["out"] for r in res.results]
    y = np.stack([outs[4 * b] + outs[4 * b + 1] + outs[4 * b + 2] + outs[4 * b + 3]
                  for b in range(B)])
    return y.astype(np.float32)


# revision 16
# speedup vs baseline: 1.3248x; 1.0839x over previous
"""Trainium2 Bass kernel for GQA attention (B=2, S=2048, D=1024, 16 q heads,
4 kv heads, head_dim 64, RoPE, causal).

Sharding: 8 cores = 2 (batch) x 4 (kv-head groups). Each core computes, for
its batch b and kv group g: the 4 query heads of group g + 1 kv head, plus the
partial output projection y_partial = attn_out_g @ wo[:, g_cols].T.  The host
unshard step sums the 4 partials per batch (the canonical all-reduce of
row-parallel TP, done on host since each core's output is already needed
host-side).

Device-side layout choices (all matmuls contract over the partition dim):
  - x is fed transposed (D on partitions) so QKV projections produce Q^T/K^T
    (head_dim on partitions, seq on free dim).
  - RoPE: wq/wk rows are permuted on host so lanes 0-31 are the "real" pair
    lanes and 32-63 the "imag" lanes; RoPE is then 2 full-width multiplies
    against replicated [c;s;c;s] tiles + 4 narrow combines on the DVE.
    (The permutation cancels in Q.K^T.)
  - Scores are computed as S^T (keys on partitions, queries on free):
    lhsT = K^T block, rhs = Q^T block.  Softmax needs no max-subtraction
    (|scores/8| <~ 3), so exp runs directly on the PSUM scores; the
    denominator is produced by an extra ones-row in the V stationary
    (out row 64 of the PV matmul = sum_l P^T[l, q]).
  - S blocks are paired into 2-bank PSUM tiles so each exp ACTIVATE covers
    (128, 1024) — halves ScalarE instruction + semaphore overhead.
  - Causal mask: matmuls are only emitted for the lower-triangle blocks; the
    128x128 diagonal blocks are masked multiplicatively (tri mask) after exp.
  - Normalization (divide by denominator, which lives along the free dim):
    reciprocal_approx_fast + gpsimd partition_broadcast + one DVE multiply.
"""

import sys

sys.path.insert(0, "/opt/trn_rl_repo")

from contextlib import ExitStack

import ml_dtypes
import numpy as np

import concourse.bass as bass
import concourse.mybir as mybir
import concourse.tile as tile
from concourse import bacc
from concourse.masks import make_identity

# ---------------------------------------------------------------- constants
B, S, D = 2, 2048, 1024
HD = 64
HALF = HD // 2
HKV = 4          # kv heads total
NH = 4           # q heads per core (= NREP)
KVD = HKV * HD   # 256
GO = NH * HD     # 256 output features per group
N_CORES = 8

SB = 512         # q superblock (matmul free dim)
NQS = S // SB    # 4 q superblocks
NLB = S // 128   # 16 key blocks of 128
KCH = D // 128   # 8 contraction chunks for projections

F32 = mybir.dt.float32
BF16 = mybir.dt.bfloat16
SCALE = 1.0 / 8.0  # 1/sqrt(64)
EXP = mybir.ActivationFunctionType.Exp

_CACHE = {}
DEBUG_DUMPS = False  # set True (before get_nc) to add intermediate outputs


# ---------------------------------------------------------------- builder
def _build():
    nc = bacc.Bacc("TRN2", target_bir_lowering=False, debug=False,
                   enable_asserts=False, num_devices=N_CORES)

    xt_d = nc.dram_tensor("xt", [D, S], BF16, kind="ExternalInput").ap()
    wqt_d = nc.dram_tensor("wqt", [D, GO], BF16, kind="ExternalInput").ap()
    wkt_d = nc.dram_tensor("wkt", [D, HD], BF16, kind="ExternalInput").ap()
    wvt_d = nc.dram_tensor("wvt", [D, HD], BF16, kind="ExternalInput").ap()
    wot_d = nc.dram_tensor("wot", [GO, D], BF16, kind="ExternalInput").ap()
    cost_d = nc.dram_tensor("cost", [HALF, S], F32, kind="ExternalInput").ap()
    sint_d = nc.dram_tensor("sint", [HALF, S], F32, kind="ExternalInput").ap()
    tri_d = nc.dram_tensor("tri", [128, 128], BF16, kind="ExternalInput").ap()
    out_d = nc.dram_tensor("out", [S, D], F32, kind="ExternalOutput").ap()
    dbg = {}
    if DEBUG_DUMPS:
        dbg["qT"] = nc.dram_tensor("dbg_qT", [128, 2, S], BF16,
                                   kind="ExternalOutput").ap()
        dbg["kT"] = nc.dram_tensor("dbg_kT", [128, S], BF16,
                                   kind="ExternalOutput").ap()
        dbg["v"] = nc.dram_tensor("dbg_v", [128, NLB, HD + 1], BF16,
                                  kind="ExternalOutput").ap()
        dbg["att"] = nc.dram_tensor("dbg_att", [128, 2, S], BF16,
                                    kind="ExternalOutput").ap()

    with ExitStack() as ctx:
        tc = ctx.enter_context(tile.TileContext(nc))
        _emit(nc, tc, ctx, xt_d, wqt_d, wkt_d, wvt_d, wot_d, cost_d, sint_d,
              tri_d, out_d, dbg)

    nc.compile()
    return nc


def _emit(nc, tc, ctx, xt_d, wqt_d, wkt_d, wvt_d, wot_d, cost_d, sint_d,
          tri_d, out_d, dbg={}):
    perm = ctx.enter_context(tc.tile_pool(name="perm", bufs=1))
    pexp = ctx.enter_context(tc.tile_pool(name="pexp", bufs=4))
    ptmp = ctx.enter_context(tc.tile_pool(name="ptmp", bufs=3))
    pout = ctx.enter_context(tc.tile_pool(name="pout", bufs=3))
    pp_mm = ctx.enter_context(tc.tile_pool(name="ppmm", bufs=2, space="PSUM"))

    # ---------------- persistent SBUF tensors
    xt_sb = perm.tile([128, KCH, S], BF16, tag="xt")
    wqt_sb = perm.tile([128, KCH, GO], BF16, tag="wqt")
    wkt_sb = perm.tile([128, KCH, HD], BF16, tag="wkt")
    wvt_sb = perm.tile([128, KCH, HD], BF16, tag="wvt")
    wot_sb = perm.tile([128, 2, D], BF16, tag="wot")
    cos4_sb = perm.tile([128, S], F32, tag="cos4")       # cos replicated 4x
    sin4_sb = perm.tile([128, S], F32, tag="sin4")       # sin replicated 4x
    tri_sb = perm.tile([128, 128], BF16, tag="tri")
    ident = perm.tile([64, 64], BF16, tag="ident")
    qT_sb = perm.tile([128, 2, S], BF16, tag="qT")       # [hd|hd, mi, s]
    kT_sb = perm.tile([128, S], BF16, tag="kT")          # rows 64-127 = dup
    v_sb = perm.tile([128, NLB, HD + 1], BF16, tag="v")  # [l, lb, hd|1]
    att_sb = perm.tile([128, 2, S], BF16, tag="att")     # [o%128, o//128, s]
    ones_sb = perm.tile([128, HD], F32, tag="ones")      # K=1 bcast stationary

    # ---------------- input DMAs
    nc.sync.dma_start(wqt_sb[:], wqt_d.rearrange("(kc p) m -> p kc m", p=128))
    nc.sync.dma_start(wkt_sb[:], wkt_d.rearrange("(kc p) m -> p kc m", p=128))
    nc.sync.dma_start(wvt_sb[:], wvt_d.rearrange("(kc p) m -> p kc m", p=128))
    nc.sync.dma_start(wot_sb[:], wot_d.rearrange("(oc p) d -> p oc d", p=128))
    for q in range(4):  # replicate cos/sin across all four 32-row groups
        nc.sync.dma_start(cos4_sb[q * 32:(q + 1) * 32, :], cost_d)
        nc.sync.dma_start(sin4_sb[q * 32:(q + 1) * 32, :], sint_d)
    nc.sync.dma_start(tri_sb[:], tri_d)
    xt_r = xt_d.rearrange("(kc p) s -> p kc s", p=128)
    for si in range(NQS):
        nc.sync.dma_start(xt_sb[:, :, si * SB:(si + 1) * SB],
                          xt_r[:, :, si * SB:(si + 1) * SB])
    make_identity(nc, ident[:])
    nc.vector.memset(ones_sb[:], 1.0)

    # ---------------- helper: RoPE on a psum projection tile
    # ps rows per 64-row head block: [real(32); imag(32)].  m0 = ps*cos in
    # SBUF, m1 = ps*sin in PSUM; each combine then mixes one SBUF operand
    # with one PSUM operand so the cross-partition pairing stays legal
    # (walrus requires all SBUF APs of a DVE op on identical partitions).
    def rope(ps, nrow, cols, dst, pool_m1):
        m0 = ptmp.tile([128, SB], BF16, tag="ropem0", name="m0")[0:nrow]
        m1 = pool_m1.tile([128, SB], F32, tag="ropem1", name="m1")[0:nrow]
        nc.vector.tensor_mul(m0[:], ps, cos4_sb[0:nrow, cols])
        nc.vector.tensor_mul(m1[:], ps, sin4_sb[0:nrow, cols])
        for b0 in range(0, nrow, 64):
            # out_r = r*c - i*s ; out_i = r*s + i*c
            nc.vector.tensor_sub(dst[b0:b0 + 32], m0[b0:b0 + 32, :],
                                 m1[b0 + 32:b0 + 64, :])
            nc.vector.tensor_add(dst[b0 + 32:b0 + 64], m1[b0:b0 + 32, :],
                                 m0[b0 + 32:b0 + 64, :])

    # ---------------- projections (stage B)
    nc.vector.memset(v_sb[:, :, HD:HD + 1], 1.0)  # ones column -> denom
    with tc.tile_pool(name="pptr", bufs=2, space="PSUM") as pp_tr:
        # Q projection + RoPE
        for mi in range(2):       # two 128-row chunks = heads (2mi, 2mi+1)
            for si in range(NQS):
                cols = slice(si * SB, (si + 1) * SB)
                ps = pp_mm.tile([128, SB], F32, tag="mm")
                for kc in range(KCH):
                    nc.tensor.matmul(
                        ps[:], wqt_sb[:, kc, mi * 128:(mi + 1) * 128],
                        xt_sb[:, kc, cols], start=(kc == 0), stop=(kc == KCH - 1))
                rope(ps[:], 128, cols, qT_sb[:, mi, cols], pp_tr)

        # K projection + RoPE
        for si in range(NQS):
            cols = slice(si * SB, (si + 1) * SB)
            psf = pp_mm.tile([128, SB], F32, tag="mm")
            ps = psf[0:64]
            for kc in range(KCH):
                nc.tensor.matmul(ps[:], wkt_sb[:, kc, :], xt_sb[:, kc, cols],
                                 start=(kc == 0), stop=(kc == KCH - 1))
            rope(ps[:], 64, cols, kT_sb[0:64, cols], pp_tr)
        # duplicate K^T to partitions 64-127 for the odd-head row-tiled matmuls
        nc.sync.dma_start(kT_sb[64:128, :], kT_sb[0:64, :])

        # V projection (V^T), then transpose to natural V
        for si in range(NQS):
            cols = slice(si * SB, (si + 1) * SB)
            psf = pp_mm.tile([128, SB], F32, tag="mm")
            ps = psf[0:64]
            for kc in range(KCH):
                nc.tensor.matmul(ps[:], wvt_sb[:, kc, :], xt_sb[:, kc, cols],
                                 start=(kc == 0), stop=(kc == KCH - 1))
            vt = ptmp.tile([64, SB], BF16, tag="vtstage")
            nc.vector.tensor_copy(vt[:], ps[:])
            for j in range(SB // 128):
                lb = si * (SB // 128) + j
                pt = pp_tr.tile([128, 64], BF16, tag="tr")
                nc.tensor.transpose(pt[:], vt[:, j * 128:(j + 1) * 128], ident[:])
                nc.vector.tensor_copy(v_sb[:, lb, 0:HD], pt[:])

    # ---------------- attention + output projection, per q superblock
    pp_sb = ctx.enter_context(tc.tile_pool(name="ppsb", bufs=2, space="PSUM"))
    pp_pv = ctx.enter_context(tc.tile_pool(name="pppv", bufs=2, space="PSUM"))

    def wo_chunk(si):
        """output projection for one 128-row q chunk"""
        scols = slice(si * 128, (si + 1) * 128)
        ysb = pout.tile([128, D], F32, tag="ysb", name="ysb")
        for dh in range(2):
            yp = pp_mm.tile([128, 512], F32, tag="mm", name="yp")
            for oc in range(2):
                nc.tensor.matmul(
                    yp[:], att_sb[:, oc, scols],
                    wot_sb[:, oc, dh * 512:(dh + 1) * 512],
                    start=(oc == 0), stop=(oc == 1))
            nc.vector.tensor_copy(ysb[:, dh * 512:(dh + 1) * 512], yp[:])
        nc.sync.dma_start(out_d[scols, :], ysb[:])

    for qs in range(NQS):
        qcols = slice(qs * SB, (qs + 1) * SB)
        nlb = 4 * qs + 4          # key blocks needed (block-causal)
        # unnormalized attention rows + denominators for this superblock are
        # staged to SBUF immediately so the PV psum slots recycle fast and
        # normalization runs off the critical path.
        db = ptmp.tile([128, SB], F32, tag="denom", name="db")
        nc.vector.memset(db[:], 1.0)  # rows 32h get denominators; rest stay 1
        aus = []
        for mi in range(2):       # head pair (2mi, 2mi+1) at partitions 0/64
            po0 = pp_pv.tile([HD + 1, SB], F32, tag="pv", name="po0")
            po1 = pp_pv.tile([HD + 1, SB], F32, tag="pv", name="po1")
            pos = (po0, po1)
            for lb in range(nlb):
                j = lb - 4 * qs   # >=0 on the diagonal superblock
                kcols = slice(lb * 128, (lb + 1) * 128)
                sp = pp_sb.tile([128, 2, SB], F32, tag="sbig", name="sp")
                # the two matmuls run concurrently (row groups 0-1 / 2-3)
                nc.tensor.matmul(sp[:, 0, :], kT_sb[0:64, kcols],
                                 qT_sb[0:64, mi, qcols], start=True, stop=True)
                nc.tensor.matmul(sp[:, 1, :], kT_sb[64:128, kcols],
                                 qT_sb[64:128, mi, qcols], start=True, stop=True)
                pe = pexp.tile([128, 2, SB], BF16, tag="pexp", name="pe")
                nc.scalar.activation(pe[:], sp[:], EXP, scale=SCALE)
                if j >= 0:
                    dcols = slice(j * 128, (j + 1) * 128)
                    nc.vector.tensor_mul(pe[:, 0, dcols], pe[:, 0, dcols],
                                         tri_sb[:])
                    nc.vector.tensor_mul(pe[:, 1, dcols], pe[:, 1, dcols],
                                         tri_sb[:])
                vcols = slice(max(j, 0) * 128, SB)
                for i in range(2):
                    nc.tensor.matmul(pos[i][:, vcols], v_sb[:, lb, :],
                                     pe[:, i, vcols],
                                     start=(lb == 0), stop=(lb == nlb - 1))
            au = ptmp.tile([128, SB], F32, tag="aun", name="au")
            aus.append(au)
            for i in range(2):
                b0 = i * 64
                r = 32 * (2 * mi + i)
                nc.vector.tensor_copy(db[r:r + 1, :], pos[i][HD:HD + 1, :])
                nc.scalar.copy(au[b0:b0 + 64, :], pos[i][0:HD, :])
            # interleave previous superblock's output projection here so the
            # TensorE has dense work while ScalarE chews the exp backlog
            if qs > 0:
                wo_chunk((qs - 1) * 4 + 2 * mi)
                wo_chunk((qs - 1) * 4 + 2 * mi + 1)
        # batched normalization for all 4 heads of this superblock: one
        # reciprocal, then per-head partition-broadcast via a K=1 outer
        # product on TensorE (PSUM operands are partition-unconstrained in
        # the final multiply).
        nc.vector.reciprocal(db[:], db[:])
        for h in range(NH):
            mi, i = h // 2, h % 2
            b0 = i * 64
            r = 32 * h
            rbp = pp_mm.tile([HD, SB], F32, tag="mm", name="rbp")
            nc.tensor.matmul(rbp[:], ones_sb[r:r + 1, :], db[r:r + 1, :],
                             start=True, stop=True,
                             tile_position=(r, 0) if r == 96 else None)
            nc.vector.tensor_mul(
                att_sb[b0:b0 + 64, mi, qcols], aus[mi][b0:b0 + 64, :],
                rbp[:])
    for sj in range(4):
        wo_chunk(3 * 4 + sj)
    if dbg:
        nc.sync.dma_start(dbg["qT"], qT_sb[:])
        nc.sync.dma_start(dbg["kT"], kT_sb[:])
        nc.sync.dma_start(dbg["v"], v_sb[:])
        nc.sync.dma_start(dbg["att"], att_sb[:])


# ---------------------------------------------------------------- host side
def _prep_inputs(x, wq, wk, wv, wo, freqs_cos, freqs_sin):
    """Shard + lay out host-side. Returns list of 8 in_maps."""
    bf = ml_dtypes.bfloat16
    # even/odd pair permutation within each head's 64 rows
    perm = np.concatenate([np.arange(0, HD, 2), np.arange(1, HD, 2)])
    cost = np.ascontiguousarray(freqs_cos.T).astype(np.float32)
    sint = np.ascontiguousarray(freqs_sin.T).astype(np.float32)
    tri = np.triu(np.ones((128, 128), np.float32)).astype(bf)  # [l, q]: l <= q

    in_maps = []
    for c in range(N_CORES):
        b, g = divmod(c, 4)
        xt = np.ascontiguousarray(x[b].T).astype(bf)
        wq_g = wq[g * GO:(g + 1) * GO].reshape(NH, HD, D)[:, perm, :].reshape(GO, D)
        wqt = np.ascontiguousarray(wq_g.T).astype(bf)
        wkt = np.ascontiguousarray(wk[g * HD:(g + 1) * HD][perm].T).astype(bf)
        wvt = np.ascontiguousarray(wv[g * HD:(g + 1) * HD].T).astype(bf)
        wot = np.ascontiguousarray(wo[:, g * GO:(g + 1) * GO].T).astype(bf)
        in_maps.append({
            "xt": xt, "wqt": wqt, "wkt": wkt, "wvt": wvt, "wot": wot,
            "cost": cost, "sint": sint, "tri": tri,
        })
    return in_maps


def get_nc():
    if "nc" not in _CACHE:
        _CACHE["nc"] = _build()
    return _CACHE["nc"]


def _ensure_ntff_hook():
    """The image's antenv lacks axon_hooks; inject an equivalent module so
    run_bass_kernel_spmd(trace=True) can capture NTFF profiles via the
    libaxon_pjrt.so C ABI (same shim trn_boot would register)."""
    import sys as _sys
    import types
    if "antenv.axon_hooks" in _sys.modules:
        return
    import contextlib
    import ctypes

    def _make_hook(so_path="/opt/axon/libaxon_pjrt.so"):
        try:
            lib = ctypes.CDLL(so_path)
        except OSError:
            return None
        if not hasattr(lib, "axon_start_nrt_profile"):
            return None
        lib.axon_start_nrt_profile.argtypes = [ctypes.POINTER(ctypes.c_int64),
                                               ctypes.c_size_t]
        lib.axon_start_nrt_profile.restype = ctypes.c_int64
        lib.axon_stop_nrt_profile.argtypes = [ctypes.c_char_p]
        lib.axon_stop_nrt_profile.restype = ctypes.c_int64

        @contextlib.contextmanager
        def _hook(output_dir, device_ids):
            import jax
            jax.devices()
            if device_ids:
                ids = (ctypes.c_int64 * len(device_ids))(*device_ids)
                rc = lib.axon_start_nrt_profile(ids, len(device_ids))
            else:
                rc = lib.axon_start_nrt_profile(None, 0)
            if rc != 0:
                raise RuntimeError(f"axon_start_nrt_profile rc={rc}")
            try:
                yield
            finally:
                n = lib.axon_stop_nrt_profile(str(output_dir).encode())
                print(f"profile: {n} file(s) -> {output_dir}", file=sys.stderr)

        return _hook

    hook = _make_hook()
    mod = types.ModuleType("antenv.axon_hooks")
    mod.get_axon_ntff_profile_hook = lambda: hook
    mod.set_axon_ntff_profile_hook = lambda h: None
    _sys.modules["antenv.axon_hooks"] = mod


def run(inputs, trace=False):
    from concourse.bass_utils import run_bass_kernel_spmd
    if trace:
        _ensure_ntff_hook()
    nc = get_nc()
    in_maps = _prep_inputs(**inputs)
    res = run_bass_kernel_spmd(nc, in_maps, core_ids=list(range(N_CORES)),
                               trace=trace)
    return res


def kernel(**inputs) -> np.ndarray:
    res = run(inputs)
    outs = [r["out"] for r in res.results]
    y = np.stack([outs[4 * b] + outs[4 * b + 1] + outs[4 * b + 2] + outs[4 * b + 3]
                  for b in range(B)])
    return y.astype(np.float32)
